# revision 23
# baseline (speedup 1.0000x reference)
"""Additive (Bahdanau) attention on 8 Trainium2 NeuronCores.

Math (per batch b, see reference):
    Qt = Q @ Wq                                  (M, H)
    Kt = K @ Wk                                  (N, H)
    scores[m,n] = sum_h wv[h] * tanh(Qt[m,h] + Kt[n,h])
    scores[m, n >= L_b] = -1e20
    out = softmax(scores, axis=-1) @ V           (M, VS)

Sharding: data-parallel over batch B=8 across the 8 cores (one batch each).

Per-core kernel design:
  - h lives on partitions: KtT_rep is (128, 512) with partitions (h, h+64)
    both holding Kt^T[h, :].  For each query *pair* (2 queries x 64 h = 128
    partitions) a DVE tensor_scalar_add broadcasts the pair's Qt values
    (a (128,1) per-partition scalar) over KtT_rep -> X tile.  X tiles for 8
    pairs are packed into one (128, 4096) tile so the ScalarE tanh runs at
    ~95% efficiency (ACT is the bottleneck engine: 16.7M tanh / core).
  - The h-reduction with wv is one PE matmul per pair: lhsT is a resident
    (128, 2) block-diag [[wv,0],[0,wv]] in float32r (1 cyc/row), rhs is the
    tanh tile (128, 512), output is 2 score rows in PSUM.
  - Key-length mask is one rank-1 matmul accumulating (-1e20) columns onto
    the PSUM scores.  Softmax needs no max-subtraction: |scores| <= sum|wv|
    (tanh is bounded), far below fp32 overflow.  Exp runs on ACT with fused
    accum_out row-sums; attn @ V uses PE-transposed E tiles in full fp32;
    the 1/rowsum scale is applied to the (128, 256) output tile.
"""

import numpy as np
from contextlib import ExitStack

import bass_rust
import concourse.bass as bass
import concourse.tile as tile
from concourse import mybir
from concourse.bass_utils import run_bass_kernel_spmd
from concourse.vector_clock import ScopedClock

B, M, N, H = 8, 512, 512, 64
QS, KS, VS = 256, 256, 256
NEG_INF = -1e20

F32 = mybir.dt.float32
F32R = mybir.dt.float32r
AF = mybir.ActivationFunctionType

PAIRS_PER_GROUP = 8           # pairs fused into one tanh instruction
GROUPS = (M // 2) // PAIRS_PER_GROUP // 4   # per m-block: 64 pairs -> 8 groups


def legalize_sync_waits(nc, max_waits=1):
    """This container's walrus build rejects instructions carrying more than
    one sync-wait command.  Tile freely attaches several.  Hoist the excess
    onto NoOp carrier instructions inserted just before, on the same engine
    (engines execute in order, so a wait moved onto an immediately-preceding
    same-engine NoOp is semantically identical)."""
    n_hoisted = 0
    for f in nc.m.functions:
        for bb in f.blocks:
            new = []
            for inst in bb.instructions:
                si = inst.sync_info
                waits = list(si.on_wait) if si is not None else []
                if len(waits) > max_waits:
                    keep = waits[-max_waits:]
                    for k, w in enumerate(waits[:-max_waits]):
                        carrier = mybir.InstNoOp(
                            name=f"{inst.name}-hoistw{k}", ins=[], outs=[])
                        carrier.engine = inst.engine
                        carrier.sync_info = type(si)(on_wait=[w], on_update=[])
                        new.append(carrier)
                        n_hoisted += 1
                    si.on_wait = keep
                new.append(inst)
            bb.instructions = new
    return n_hoisted


X_BF16 = False       # bf16 X tiles: faster DVE adds but ACT tanh runs ~17% slower
KT_BF16 = False      # bf16 ktT_rep: DVE adds at 2x (~326ns) at ~1e-3 accuracy cost
GP_PER_GROUP = 2     # tensor_scalar adds per group routed to GpSimd (0..8)
FP32R_LOADS = True   # declare matmul-operand DRAM inputs as float32r


def build_nc():
    nc = bass.Bass("TRN2", target_bir_lowering=False, debug=False, num_devices=8)

    LD = F32R if FP32R_LOADS else F32
    XDT = mybir.dt.bfloat16 if X_BF16 else F32

    qT = nc.dram_tensor("qT", (QS, M), LD, kind="ExternalInput")
    kT = nc.dram_tensor("kT", (KS, N), LD, kind="ExternalInput")
    v = nc.dram_tensor("v", (N, VS), LD, kind="ExternalInput")
    wq = nc.dram_tensor("wq", (QS, H), LD, kind="ExternalInput")
    wk = nc.dram_tensor("wk", (KS, H), LD, kind="ExternalInput")
    wvb = nc.dram_tensor("wvb", (128, 32, 64), LD, kind="ExternalInput")
    maskneg = nc.dram_tensor("maskneg", (1, N), LD, kind="ExternalInput")
    ones_r = nc.dram_tensor("ones_r", (1, 64), LD, kind="ExternalInput")
    ident = nc.dram_tensor("ident", (64, 64), F32, kind="ExternalInput")
    out = nc.dram_tensor("out", (M, VS), F32, kind="ExternalOutput")

    with tile.TileContext(nc) as tc:
        with ExitStack() as ctx:
            singles = ctx.enter_context(tc.tile_pool(name="singles", bufs=1))

            # warm the ACT table set (exp_and_others covers tanh+exp) at t=0
            warm = singles.tile([1, 1], F32)
            nc.vector.memset(warm, 0.0)
            nc.scalar.activation(warm, warm, AF.Tanh)

            # ---- loads needed by the projections first ----
            wq_sb = singles.tile([128, QS // 128, H], LD)
            nc.sync.dma_start(out=wq_sb, in_=wq.ap().rearrange("(kt p) h -> p kt h", p=128))
            wk_sb = singles.tile([128, KS // 128, H], LD)
            nc.sync.dma_start(out=wk_sb, in_=wk.ap().rearrange("(kt p) h -> p kt h", p=128))

            # ---- projections ----
            # qpairs: partitions 0-63 hold Qt^T, partitions 64-127 hold Qt^T
            # shifted left one column, so qpairs[:, 2j] is the per-partition
            # bias [Qt[2j, :]; Qt[2j+1, :]] for pair j.
            with tc.tile_pool(name="setup_sb", bufs=1) as setup_sb, \
                 tc.tile_pool(name="setup_ps", bufs=2, space="PSUM") as setup_ps:
                qT_sb = setup_sb.tile([128, QS // 128, M], LD)
                nc.sync.dma_start(out=qT_sb, in_=qT.ap().rearrange("(kt p) m -> p kt m", p=128))
                kT_sb = setup_sb.tile([128, KS // 128, N], LD)
                nc.sync.dma_start(out=kT_sb, in_=kT.ap().rearrange("(kt p) m -> p kt m", p=128))

                # Qt^T at PSUM base 0 (fp32r matmuls may only write base 0),
                # then build the pair-bias layout in SBUF: top half = Qt^T,
                # bottom half = Qt^T shifted one column (via SBUF->SBUF DMA,
                # which has no partition-base restrictions).
                ps_q = setup_ps.tile([64, M], F32)
                for kt in range(QS // 128):
                    nc.tensor.matmul(ps_q, wq_sb[:, kt, :], qT_sb[:, kt, :],
                                     start=(kt == 0), stop=(kt == QS // 128 - 1))
                qpairs = singles.tile([128, M], F32)
                nc.vector.tensor_copy(qpairs[0:64, :], ps_q)
                nc.sync.dma_start(out=qpairs[64:128, 0:M - 1], in_=qpairs[0:64, 1:M])

                ps_k = setup_ps.tile([64, N], F32)
                for kt in range(KS // 128):
                    nc.tensor.matmul(ps_k, wk_sb[:, kt, :], kT_sb[:, kt, :],
                                     start=(kt == 0), stop=(kt == KS // 128 - 1))
                ktT_rep = singles.tile([128, N], mybir.dt.bfloat16 if KT_BF16 else F32)
                nc.vector.tensor_copy(ktT_rep[0:64, :], ps_k)
                nc.sync.dma_start(out=ktT_rep[64:128, :], in_=ktT_rep[0:64, :])

            # ---- remaining static loads (off the critical path) ----
            v_sb = singles.tile([128, N // 128, VS], LD)
            nc.sync.dma_start(out=v_sb, in_=v.ap().rearrange("(nt p) c -> p nt c", p=128))
            wvb_sb = singles.tile([128, 32, 64], LD)
            nc.sync.dma_start(out=wvb_sb, in_=wvb.ap())
            if FP32R_LOADS:
                wvb_r = wvb_sb
            else:
                wvb_r = singles.tile([128, 32, 64], F32R)
                nc.vector.tensor_copy(wvb_r, wvb_sb)
            mask_sb = singles.tile([1, N], LD)
            nc.sync.dma_start(out=mask_sb, in_=maskneg.ap())
            ones_sb = singles.tile([1, 64], LD)
            nc.sync.dma_start(out=ones_sb, in_=ones_r.ap())
            ident_sb = singles.tile([64, 64], F32)
            nc.sync.dma_start(out=ident_sb, in_=ident.ap())

            # ---- pools for the main loop ----
            xpool = ctx.enter_context(tc.tile_pool(name="x", bufs=3))
            tpool = ctx.enter_context(tc.tile_pool(name="t", bufs=2))
            epool = ctx.enter_context(tc.tile_pool(name="e", bufs=2))
            etpool = ctx.enter_context(tc.tile_pool(name="et", bufs=2))
            opool = ctx.enter_context(tc.tile_pool(name="o", bufs=2))
            stats = ctx.enter_context(tc.tile_pool(name="stats", bufs=4))
            ps_sc_pool = ctx.enter_context(tc.tile_pool(name="ps_sc", bufs=2, space="PSUM"))
            ps_tr_pool = ctx.enter_context(tc.tile_pool(name="ps_tr", bufs=2, space="PSUM"))
            ps_o_pool = ctx.enter_context(tc.tile_pool(name="ps_o", bufs=2, space="PSUM"))

            st = {}  # per-(mb, half) pipeline state

            def emit_group(mb, g):
                m0 = mb * 128
                x = xpool.tile([128, PAIRS_PER_GROUP, N], XDT, name=f"x_{mb}_{g}", tag="x")
                for j in range(PAIRS_PER_GROUP):
                    c = m0 + 2 * (g * PAIRS_PER_GROUP + j)
                    eng = nc.gpsimd if j >= PAIRS_PER_GROUP - GP_PER_GROUP else nc.vector
                    eng.tensor_scalar_add(x[:, j, :], ktT_rep, qpairs[:, c:c + 1])
                t = tpool.tile([128, PAIRS_PER_GROUP, N], F32R, name=f"t_{mb}_{g}", tag="t")
                nc.scalar.activation(t, x, AF.Tanh)
                for j in range(PAIRS_PER_GROUP):
                    p = g * PAIRS_PER_GROUP + j      # pair index 0..63
                    q, jj = divmod(p, 32)            # half, slot in half
                    # block-diag wv variant jj writes rows (2jj, 2jj+1) of
                    # half q; other rows accumulate zeros.
                    nc.tensor.matmul(st[(mb, q)], wvb_r[:, jj, :], t[:, j, :],
                                     start=(jj == 0), stop=(jj == 31))

            def emit_mask_exp(mb, q):
                s = st[(mb, q)]
                # rank-1 additive mask onto all 64 score rows of the half
                nc.tensor.matmul(s, ones_sb, mask_sb, start=False, stop=True)
                e = epool.tile([64, N], F32, name=f"e_{mb}_{q}", tag=f"e{q}")
                rsum = stats.tile([64, 1], F32, name=f"rsum_{mb}_{q}", tag=f"rsum{q}")
                nc.scalar.activation(e, s, AF.Exp, accum_out=rsum)
                st[(mb, q, "e")] = e
                st[(mb, q, "rsum")] = rsum

            def emit_transposes(mb, q):
                e = st[(mb, q, "e")]
                et = etpool.tile([128, N // 128, 64], F32R, name=f"et_{mb}_{q}", tag=f"et{q}")
                for nt in range(N // 128):
                    ps_tr = ps_tr_pool.tile([128, 64], F32, name=f"tr_{mb}_{q}_{nt}", tag="tr")
                    nc.tensor.transpose(ps_tr, e[:, nt * 128:(nt + 1) * 128], ident_sb)
                    nc.vector.tensor_copy(et[:, nt, :], ps_tr)
                st[(mb, q, "et")] = et

            def emit_out(mb, q):
                m0 = mb * 128
                et = st.pop((mb, q, "et"))
                rsum = st.pop((mb, q, "rsum"))
                st.pop((mb, q, "e"))
                st.pop((mb, q))
                rinv = stats.tile([64, 1], F32, name=f"rinv_{mb}_{q}", tag=f"rinv{q}")
                nc.vector.reciprocal(rinv, rsum)
                ps_o = ps_o_pool.tile([64, VS], F32, name=f"ps_o_{mb}_{q}", tag="po")
                for nt in range(N // 128):
                    nc.tensor.matmul(ps_o, et[:, nt, :], v_sb[:, nt, :],
                                     start=(nt == 0), stop=(nt == N // 128 - 1))
                o_sb = opool.tile([64, VS], F32, name=f"o_{mb}_{q}", tag=f"o{q}")
                nc.vector.tensor_scalar_mul(o_sb, ps_o, rinv)
                nc.sync.dma_start(out=out[m0 + 64 * q:m0 + 64 * q + 64, :], in_=o_sb)

            # Software-pipelined emission.  Half q of block mb has all its
            # score matmuls emitted by the end of group 4q+3; its
            # post-processing is spread over the following groups (2 groups
            # of slack per stage) so no engine head-of-line-blocks on
            # cross-engine waits while tanh work remains.
            NG = GROUPS  # 8 groups per mb; halves complete after g3 / g7
            total = (M // 128) * NG
            for k in range(total):
                mb, g = divmod(k, NG)
                if g == 0:
                    st[(mb, 0)] = ps_sc_pool.tile([64, N], F32, name=f"ps_h_{mb}_0", tag="ps_h0")
                    st[(mb, 1)] = ps_sc_pool.tile([64, N], F32, name=f"ps_h_{mb}_1", tag="ps_h1")
                emit_group(mb, g)
                # half hi = (mb*2+q) finishes its reduce MMs at group
                # 8*mb+4q+3; stages run 1/3/5 groups later.
                hidx = k - 4          # mask+exp for half hidx//4 when hidx%4==0
                if hidx >= 0 and hidx % 4 == 0:
                    emit_mask_exp(hidx // 8, (hidx // 4) % 2)
                hidx = k - 6
                if hidx >= 0 and hidx % 4 == 0:
                    emit_transposes(hidx // 8, (hidx // 4) % 2)
                hidx = k - 8
                if hidx >= 0 and hidx % 4 == 0:
                    emit_out(hidx // 8, (hidx // 4) % 2)
            # drain the remaining pipeline stages
            emit_mask_exp(M // 128 - 1, 1)
            emit_transposes(M // 128 - 1, 1)
            emit_out(M // 128 - 1, 0)
            emit_out(M // 128 - 1, 1)

    legalize_sync_waits(nc)
    return nc


_NC_CACHE = []


def _get_nc():
    if not _NC_CACHE:
        _NC_CACHE.append(build_nc())
    return _NC_CACHE[0]


def make_in_maps(Q, K, V, valid_lens, Wq, Wk, wv):
    Q = np.asarray(Q, np.float32)
    K = np.asarray(K, np.float32)
    V = np.asarray(V, np.float32)
    Wq = np.asarray(Wq, np.float32)
    Wk = np.asarray(Wk, np.float32)
    wv = np.asarray(wv, np.float32)
    lens = np.asarray(valid_lens).astype(np.int64)

    wvb = np.zeros((128, 32, 64), np.float32)
    for j in range(32):
        wvb[0:64, j, 2 * j] = wv
        wvb[64:128, j, 2 * j + 1] = wv
    ones_r = np.ones((1, 64), np.float32)
    ident = np.eye(64, dtype=np.float32)

    in_maps = []
    for b in range(B):
        maskneg = np.zeros((1, N), np.float32)
        maskneg[0, int(lens[b]):] = NEG_INF
        in_maps.append({
            "qT": np.ascontiguousarray(Q[b].T),
            "kT": np.ascontiguousarray(K[b].T),
            "v": np.ascontiguousarray(V[b]),
            "wq": Wq, "wk": Wk, "wvb": wvb,
            "maskneg": maskneg, "ones_r": ones_r, "ident": ident,
        })
    return in_maps


def run(Q, K, V, valid_lens, Wq, Wk, wv, trace=False, **kw):
    nc = _get_nc()
    in_maps = make_in_maps(Q, K, V, valid_lens, Wq, Wk, wv)
    res = run_bass_kernel_spmd(nc, in_maps, core_ids=list(range(B)), trace=trace, **kw)
    out = np.stack([res.results[b]["out"] for b in range(B)]).astype(np.float32)
    return out, res


def kernel(Q, K, V, valid_lens, Wq, Wk, wv):
    out, _ = run(Q, K, V, valid_lens, Wq, Wk, wv, trace=False)
    return out


# revision 24
# speedup vs baseline: 4.4578x; 4.4578x over previous
"""Additive (Bahdanau) attention on 8 Trainium2 NeuronCores.

Math (per batch b, see reference):
    Qt = Q @ Wq                                  (M, H)
    Kt = K @ Wk                                  (N, H)
    scores[m,n] = sum_h wv[h] * tanh(Qt[m,h] + Kt[n,h])
    scores[m, n >= L_b] = -1e20
    out = softmax(scores, axis=-1) @ V           (M, VS)

Sharding: data-parallel over batch B=8 across the 8 cores (one batch each).

Per-core kernel design:
  - h lives on partitions: KtT_rep is (128, 512) with partitions (h, h+64)
    both holding Kt^T[h, :].  For each query *pair* (2 queries x 64 h = 128
    partitions) a DVE tensor_scalar_add broadcasts the pair's Qt values
    (a (128,1) per-partition scalar) over KtT_rep -> X tile.  X tiles for 8
    pairs are packed into one (128, 4096) tile so the ScalarE tanh runs at
    ~95% efficiency (ACT is the bottleneck engine: 16.7M tanh / core).
  - The h-reduction with wv is one PE matmul per pair: lhsT is a resident
    (128, 2) block-diag [[wv,0],[0,wv]] in float32r (1 cyc/row), rhs is the
    tanh tile (128, 512), output is 2 score rows in PSUM.
  - Key-length mask is one rank-1 matmul accumulating (-1e20) columns onto
    the PSUM scores.  Softmax needs no max-subtraction: |scores| <= sum|wv|
    (tanh is bounded), far below fp32 overflow.  Exp runs on ACT with fused
    accum_out row-sums; attn @ V uses PE-transposed E tiles in full fp32;
    the 1/rowsum scale is applied to the (128, 256) output tile.
"""

import numpy as np
from contextlib import ExitStack

import bass_rust
import concourse.bass as bass
import concourse.tile as tile
from concourse import mybir
from concourse.bass_utils import run_bass_kernel_spmd
from concourse.vector_clock import ScopedClock

B, M, N, H = 8, 512, 512, 64
QS, KS, VS = 256, 256, 256
NEG_INF = -1e20

F32 = mybir.dt.float32
F32R = mybir.dt.float32r
AF = mybir.ActivationFunctionType

PAIRS_PER_GROUP = 8           # pairs fused into one tanh instruction
GROUPS = (M // 2) // PAIRS_PER_GROUP // 4   # per m-block: 64 pairs -> 8 groups


def legalize_sync_waits(nc, max_waits=1):
    """This container's walrus build rejects instructions carrying more than
    one sync-wait command.  Tile freely attaches several.  Hoist the excess
    onto NoOp carrier instructions inserted just before, on the same engine
    (engines execute in order, so a wait moved onto an immediately-preceding
    same-engine NoOp is semantically identical)."""
    n_hoisted = 0
    for f in nc.m.functions:
        for bb in f.blocks:
            new = []
            for inst in bb.instructions:
                si = inst.sync_info
                waits = list(si.on_wait) if si is not None else []
                if len(waits) > max_waits:
                    keep = waits[-max_waits:]
                    for k, w in enumerate(waits[:-max_waits]):
                        carrier = mybir.InstNoOp(
                            name=f"{inst.name}-hoistw{k}", ins=[], outs=[])
                        carrier.engine = inst.engine
                        carrier.sync_info = type(si)(on_wait=[w], on_update=[])
                        new.append(carrier)
                        n_hoisted += 1
                    si.on_wait = keep
                new.append(inst)
            bb.instructions = new
    return n_hoisted


X_BF16 = False       # bf16 X tiles: faster DVE adds but ACT tanh runs ~17% slower
KT_BF16 = False      # bf16 ktT_rep: DVE adds at 2x (~326ns) at ~1e-3 accuracy cost
GP_PER_GROUP = 0     # tensor_scalar adds per group routed to GpSimd (slow: ~9.4us each, and SBUF port contention slows DVE ~8x - keep 0)
FP32R_LOADS = True   # declare matmul-operand DRAM inputs as float32r


def build_nc():
    nc = bass.Bass("TRN2", target_bir_lowering=False, debug=False, num_devices=8)

    LD = F32R if FP32R_LOADS else F32
    XDT = mybir.dt.bfloat16 if X_BF16 else F32

    qT = nc.dram_tensor("qT", (QS, M), LD, kind="ExternalInput")
    kT = nc.dram_tensor("kT", (KS, N), LD, kind="ExternalInput")
    v = nc.dram_tensor("v", (N, VS), LD, kind="ExternalInput")
    wq = nc.dram_tensor("wq", (QS, H), LD, kind="ExternalInput")
    wk = nc.dram_tensor("wk", (KS, H), LD, kind="ExternalInput")
    wvb = nc.dram_tensor("wvb", (128, 32, 64), LD, kind="ExternalInput")
    maskneg = nc.dram_tensor("maskneg", (1, N), LD, kind="ExternalInput")
    ones_r = nc.dram_tensor("ones_r", (1, 64), LD, kind="ExternalInput")
    ident = nc.dram_tensor("ident", (64, 64), F32, kind="ExternalInput")
    out = nc.dram_tensor("out", (M, VS), F32, kind="ExternalOutput")

    with tile.TileContext(nc) as tc:
        with ExitStack() as ctx:
            singles = ctx.enter_context(tc.tile_pool(name="singles", bufs=1))

            # warm the ACT table set (exp_and_others covers tanh+exp) at t=0
            warm = singles.tile([1, 1], F32)
            nc.vector.memset(warm, 0.0)
            nc.scalar.activation(warm, warm, AF.Tanh)

            # ---- loads needed by the projections first ----
            wq_sb = singles.tile([128, QS // 128, H], LD)
            nc.sync.dma_start(out=wq_sb, in_=wq.ap().rearrange("(kt p) h -> p kt h", p=128))
            wk_sb = singles.tile([128, KS // 128, H], LD)
            nc.sync.dma_start(out=wk_sb, in_=wk.ap().rearrange("(kt p) h -> p kt h", p=128))

            # ---- projections ----
            # qpairs: partitions 0-63 hold Qt^T, partitions 64-127 hold Qt^T
            # shifted left one column, so qpairs[:, 2j] is the per-partition
            # bias [Qt[2j, :]; Qt[2j+1, :]] for pair j.
            with tc.tile_pool(name="setup_sb", bufs=1) as setup_sb, \
                 tc.tile_pool(name="setup_ps", bufs=2, space="PSUM") as setup_ps:
                qT_sb = setup_sb.tile([128, QS // 128, M], LD)
                nc.sync.dma_start(out=qT_sb, in_=qT.ap().rearrange("(kt p) m -> p kt m", p=128))
                kT_sb = setup_sb.tile([128, KS // 128, N], LD)
                nc.sync.dma_start(out=kT_sb, in_=kT.ap().rearrange("(kt p) m -> p kt m", p=128))

                # Qt^T at PSUM base 0 (fp32r matmuls may only write base 0),
                # then build the pair-bias layout in SBUF: top half = Qt^T,
                # bottom half = Qt^T shifted one column (via SBUF->SBUF DMA,
                # which has no partition-base restrictions).
                ps_q = setup_ps.tile([64, M], F32)
                for kt in range(QS // 128):
                    nc.tensor.matmul(ps_q, wq_sb[:, kt, :], qT_sb[:, kt, :],
                                     start=(kt == 0), stop=(kt == QS // 128 - 1))
                qpairs = singles.tile([128, M], F32)
                nc.vector.tensor_copy(qpairs[0:64, :], ps_q)
                nc.sync.dma_start(out=qpairs[64:128, 0:M - 1], in_=qpairs[0:64, 1:M])

                ps_k = setup_ps.tile([64, N], F32)
                for kt in range(KS // 128):
                    nc.tensor.matmul(ps_k, wk_sb[:, kt, :], kT_sb[:, kt, :],
                                     start=(kt == 0), stop=(kt == KS // 128 - 1))
                ktT_rep = singles.tile([128, N], mybir.dt.bfloat16 if KT_BF16 else F32)
                nc.vector.tensor_copy(ktT_rep[0:64, :], ps_k)
                nc.sync.dma_start(out=ktT_rep[64:128, :], in_=ktT_rep[0:64, :])

            # ---- remaining static loads (off the critical path) ----
            v_sb = singles.tile([128, N // 128, VS], LD)
            nc.sync.dma_start(out=v_sb, in_=v.ap().rearrange("(nt p) c -> p nt c", p=128))
            wvb_sb = singles.tile([128, 32, 64], LD)
            nc.sync.dma_start(out=wvb_sb, in_=wvb.ap())
            if FP32R_LOADS:
                wvb_r = wvb_sb
            else:
                wvb_r = singles.tile([128, 32, 64], F32R)
                nc.vector.tensor_copy(wvb_r, wvb_sb)
            mask_sb = singles.tile([1, N], LD)
            nc.sync.dma_start(out=mask_sb, in_=maskneg.ap())
            ones_sb = singles.tile([1, 64], LD)
            nc.sync.dma_start(out=ones_sb, in_=ones_r.ap())
            ident_sb = singles.tile([64, 64], F32)
            nc.sync.dma_start(out=ident_sb, in_=ident.ap())

            # ---- pools for the main loop ----
            xpool = ctx.enter_context(tc.tile_pool(name="x", bufs=3))
            tpool = ctx.enter_context(tc.tile_pool(name="t", bufs=2))
            epool = ctx.enter_context(tc.tile_pool(name="e", bufs=2))
            etpool = ctx.enter_context(tc.tile_pool(name="et", bufs=2))
            opool = ctx.enter_context(tc.tile_pool(name="o", bufs=2))
            stats = ctx.enter_context(tc.tile_pool(name="stats", bufs=4))
            ps_sc_pool = ctx.enter_context(tc.tile_pool(name="ps_sc", bufs=2, space="PSUM"))
            ps_tr_pool = ctx.enter_context(tc.tile_pool(name="ps_tr", bufs=2, space="PSUM"))
            ps_o_pool = ctx.enter_context(tc.tile_pool(name="ps_o", bufs=2, space="PSUM"))

            st = {}  # per-(mb, half) pipeline state

            def emit_group(mb, g):
                m0 = mb * 128
                x = xpool.tile([128, PAIRS_PER_GROUP, N], XDT, name=f"x_{mb}_{g}", tag="x")
                for j in range(PAIRS_PER_GROUP):
                    c = m0 + 2 * (g * PAIRS_PER_GROUP + j)
                    eng = nc.gpsimd if j >= PAIRS_PER_GROUP - GP_PER_GROUP else nc.vector
                    eng.tensor_scalar_add(x[:, j, :], ktT_rep, qpairs[:, c:c + 1])
                t = tpool.tile([128, PAIRS_PER_GROUP, N], F32R, name=f"t_{mb}_{g}", tag="t")
                nc.scalar.activation(t, x, AF.Tanh)
                for j in range(PAIRS_PER_GROUP):
                    p = g * PAIRS_PER_GROUP + j      # pair index 0..63
                    q, jj = divmod(p, 32)            # half, slot in half
                    # block-diag wv variant jj writes rows (2jj, 2jj+1) of
                    # half q; other rows accumulate zeros.
                    nc.tensor.matmul(st[(mb, q)], wvb_r[:, jj, :], t[:, j, :],
                                     start=(jj == 0), stop=(jj == 31))

            def emit_mask_exp(mb, q):
                s = st[(mb, q)]
                # rank-1 additive mask onto all 64 score rows of the half
                nc.tensor.matmul(s, ones_sb, mask_sb, start=False, stop=True)
                e = epool.tile([64, N], F32, name=f"e_{mb}_{q}", tag=f"e{q}")
                rsum = stats.tile([64, 1], F32, name=f"rsum_{mb}_{q}", tag=f"rsum{q}")
                nc.scalar.activation(e, s, AF.Exp, accum_out=rsum)
                st[(mb, q, "e")] = e
                st[(mb, q, "rsum")] = rsum

            def emit_transposes(mb, q):
                e = st[(mb, q, "e")]
                et = etpool.tile([128, N // 128, 64], F32R, name=f"et_{mb}_{q}", tag=f"et{q}")
                for nt in range(N // 128):
                    ps_tr = ps_tr_pool.tile([128, 64], F32, name=f"tr_{mb}_{q}_{nt}", tag="tr")
                    nc.tensor.transpose(ps_tr, e[:, nt * 128:(nt + 1) * 128], ident_sb)
                    nc.vector.tensor_copy(et[:, nt, :], ps_tr)
                st[(mb, q, "et")] = et

            def emit_out(mb, q):
                m0 = mb * 128
                et = st.pop((mb, q, "et"))
                rsum = st.pop((mb, q, "rsum"))
                st.pop((mb, q, "e"))
                st.pop((mb, q))
                rinv = stats.tile([64, 1], F32, name=f"rinv_{mb}_{q}", tag=f"rinv{q}")
                nc.vector.reciprocal(rinv, rsum)
                ps_o = ps_o_pool.tile([64, VS], F32, name=f"ps_o_{mb}_{q}", tag="po")
                for nt in range(N // 128):
                    nc.tensor.matmul(ps_o, et[:, nt, :], v_sb[:, nt, :],
                                     start=(nt == 0), stop=(nt == N // 128 - 1))
                o_sb = opool.tile([64, VS], F32, name=f"o_{mb}_{q}", tag=f"o{q}")
                nc.vector.tensor_scalar_mul(o_sb, ps_o, rinv)
                nc.sync.dma_start(out=out[m0 + 64 * q:m0 + 64 * q + 64, :], in_=o_sb)

            # Software-pipelined emission.  Half q of block mb has all its
            # score matmuls emitted by the end of group 4q+3; its
            # post-processing is spread over the following groups (2 groups
            # of slack per stage) so no engine head-of-line-blocks on
            # cross-engine waits while tanh work remains.
            NG = GROUPS  # 8 groups per mb; halves complete after g3 / g7
            total = (M // 128) * NG
            for k in range(total):
                mb, g = divmod(k, NG)
                if g == 0:
                    st[(mb, 0)] = ps_sc_pool.tile([64, N], F32, name=f"ps_h_{mb}_0", tag="ps_h0")
                    st[(mb, 1)] = ps_sc_pool.tile([64, N], F32, name=f"ps_h_{mb}_1", tag="ps_h1")
                emit_group(mb, g)
                # half hi = (mb*2+q) finishes its reduce MMs at group
                # 8*mb+4q+3; stages run 1/3/5 groups later.
                hidx = k - 4          # mask+exp for half hidx//4 when hidx%4==0
                if hidx >= 0 and hidx % 4 == 0:
                    emit_mask_exp(hidx // 8, (hidx // 4) % 2)
                hidx = k - 6
                if hidx >= 0 and hidx % 4 == 0:
                    emit_transposes(hidx // 8, (hidx // 4) % 2)
                hidx = k - 8
                if hidx >= 0 and hidx % 4 == 0:
                    emit_out(hidx // 8, (hidx // 4) % 2)
            # drain the remaining pipeline stages
            emit_mask_exp(M // 128 - 1, 1)
            emit_transposes(M // 128 - 1, 1)
            emit_out(M // 128 - 1, 0)
            emit_out(M // 128 - 1, 1)

    legalize_sync_waits(nc)
    return nc


_NC_CACHE = []


def _get_nc():
    if not _NC_CACHE:
        _NC_CACHE.append(build_nc())
    return _NC_CACHE[0]


def make_in_maps(Q, K, V, valid_lens, Wq, Wk, wv):
    Q = np.asarray(Q, np.float32)
    K = np.asarray(K, np.float32)
    V = np.asarray(V, np.float32)
    Wq = np.asarray(Wq, np.float32)
    Wk = np.asarray(Wk, np.float32)
    wv = np.asarray(wv, np.float32)
    lens = np.asarray(valid_lens).astype(np.int64)

    wvb = np.zeros((128, 32, 64), np.float32)
    for j in range(32):
        wvb[0:64, j, 2 * j] = wv
        wvb[64:128, j, 2 * j + 1] = wv
    ones_r = np.ones((1, 64), np.float32)
    ident = np.eye(64, dtype=np.float32)

    in_maps = []
    for b in range(B):
        maskneg = np.zeros((1, N), np.float32)
        maskneg[0, int(lens[b]):] = NEG_INF
        in_maps.append({
            "qT": np.ascontiguousarray(Q[b].T),
            "kT": np.ascontiguousarray(K[b].T),
            "v": np.ascontiguousarray(V[b]),
            "wq": Wq, "wk": Wk, "wvb": wvb,
            "maskneg": maskneg, "ones_r": ones_r, "ident": ident,
        })
    return in_maps


def run(Q, K, V, valid_lens, Wq, Wk, wv, trace=False, **kw):
    nc = _get_nc()
    in_maps = make_in_maps(Q, K, V, valid_lens, Wq, Wk, wv)
    res = run_bass_kernel_spmd(nc, in_maps, core_ids=list(range(B)), trace=trace, **kw)
    out = np.stack([res.results[b]["out"] for b in range(B)]).astype(np.float32)
    return out, res


def kernel(Q, K, V, valid_lens, Wq, Wk, wv):
    out, _ = run(Q, K, V, valid_lens, Wq, Wk, wv, trace=False)
    return out
